# revision 53
# baseline (speedup 1.0000x reference)
"""ClusterGCN layer on 8 TRN2 NeuronCores.

Math per cluster c (only intra-cluster edges matter):
    Y_c = D^-1/2 (A_c + I) D^-1/2 X_c W + b,  D = intra-degree + 1.
Host pre-scales X rows by dis = rsqrt(deg) and ships integer edge
counts At (exact in fp8e4 for counts <= 16); device computes
    xws = Xs @ W                      (nodes on partitions)
    Z^T[f, d] = sum_s xws[s, f] * At_c[s, d]
host applies dis[d] + bias on gather and passes through clusters with
no intra edges.

Device schedule: every input gets its own SBUF tile (bufs=cpc pools,
no reuse) and ALL load DMAs are issued up front on the single sync
queue in consumption order (XW0, X1, A0, X2, A1, ...; X one cluster
ahead of At) -- the queue FIFO gives the critical first tiles full
DMA bandwidth and later loads always run ahead of the PE, so the
matmul stream (measured at the full 2.4GHz once fed) never stalls.
W rides in cluster 0's row tails (XW0) so the cold-clock startup
window moves one packed stream.  Eight 512-col warmup matmuls on a
zeroed tile bridge the PE through its DVFS ramp (0.65/1.2/2.4 GHz,
~3us of continuous work to reach full clock) until the first data
lands at ~11us -- the chip-wide cold-DMA ramp makes that floor.
Per cluster: step1 Xs@W into 2 PSUM banks (scalar + vector casts;
vector-only for clusters 0-1 since scalar's first ACTIVATE waits on
its cold activation-table DMA), then step2 f x d matmuls vs fp8 At
(drains alternate vector/scalar), one 250KB YT store per cluster on
the scalar queue.  Software pipeline: step1(c+1) is emitted before
step2(c) so cluster-boundary casts never block the PE.  The last
cluster uses 256-wide d-chunks and per-f stores on two queues so the
final drain+store tail is short.  The remaining fixed costs (~6us
BSP prologue excluded from exec_time, ~9us semaphore-clear teardown
counted in it) are framework constants.
"""

import numpy as np

N_CORES = 8
N_CLUSTERS = 100
P = 128
N_WARM = 8
WARM_COLS = 512

_prog_cache: dict = {}


def _build_program(cpc: int, cap: int, dcap: int, in_c: int, f_out: int,
                   a_fp8: bool):
    import concourse.mybir as mybir
    import concourse.tile as tile
    from concourse import bacc

    key = (cpc, cap, dcap, in_c, f_out, a_fp8)
    if key in _prog_cache:
        return _prog_cache[key]

    kc = in_c // P           # contraction chunks for X @ W
    sch = cap // P           # s-tiles per cluster
    fc = f_out // P          # f chunks (step-2 output partitions)
    hs = sch // 2            # merged step-1 PSUM banks (2 s-tiles each)
    f32 = mybir.dt.float32
    x_dt = mybir.dt.float16
    a_dt = mybir.dt.float8e4 if a_fp8 else x_dt

    nc = bacc.Bacc("TRN2", target_bir_lowering=False, debug=False,
                   num_devices=N_CORES)

    XT = nc.dram_tensor("XT", [P, cpc, kc, cap], x_dt, kind="ExternalInput")
    # cluster 0's X with W packed into the row tail: one cold-window DMA
    XW0 = nc.dram_tensor("XW0", [P, kc, cap + f_out], x_dt,
                         kind="ExternalInput")
    AT = nc.dram_tensor("AT", [P, cpc, sch * dcap], a_dt, kind="ExternalInput")
    YT = nc.dram_tensor("YT", [cpc, P, fc, dcap], x_dt, kind="ExternalOutput")

    with tile.TileContext(nc) as tc:
        with (
            tc.tile_pool(name="wm", bufs=2) as wm_pool,
            tc.tile_pool(name="w", bufs=2) as w_pool,
            tc.tile_pool(name="xc", bufs=cpc) as xc_pool,
            tc.tile_pool(name="ac", bufs=cpc) as ac_pool,
            tc.tile_pool(name="xw", bufs=6) as xw_pool,
            tc.tile_pool(name="ot", bufs=6) as ot_pool,
            tc.tile_pool(name="ps1", bufs=4, space="PSUM") as ps1_pool,
            tc.tile_pool(name="ps2", bufs=4, space="PSUM") as ps2_pool,
        ):
            # warmup tile in its own pool (sharing a bufs=1 pool with wt
            # would make the W load wait on every warmup matmul via WAR)
            warm = wm_pool.tile([P, WARM_COLS], x_dt)
            nc.gpsimd.memset(warm[:], 0)
            # ALL input loads on the sync queue, strictly in consumption
            # order (XW0, A0, X1, A1, ...): queue FIFO guarantees the
            # critical first tiles get full DMA bandwidth instead of
            # round-robin-sharing it with bulk prefetch on other queues.
            # Per-cluster tiles, no reuse -> loads never wait on compute.
            # W rides in cluster 0's row tails (XW0) so the cold window
            # moves one packed stream instead of X0 + W separately.
            xw0 = w_pool.tile([P, kc, cap + f_out], x_dt)
            wt = xw0[:, :, cap:]
            xc = [xw0] + [xc_pool.tile([P, kc, cap], x_dt, name="xct")
                          for _ in range(cpc - 1)]
            ac = [ac_pool.tile([P, sch * dcap], a_dt, name="act")
                  for _ in range(cpc)]
            # X runs one cluster ahead of At since step1(c+1) precedes
            # step2(c)
            nc.sync.dma_start(xw0[:], XW0[:])
            for c in range(cpc):
                if c + 1 < cpc:
                    nc.sync.dma_start(xc[c + 1][:], XT[:, c + 1])
                nc.sync.dma_start(ac[c][:], AT[:, c])
            for _ in range(N_WARM):
                psw = ps2_pool.tile([P, 512], f32, name="ps2t")
                nc.tensor.matmul(psw[:, :WARM_COLS], lhsT=warm[:, :P],
                                 rhs=warm[:],
                                 start=True, stop=True)

            def step2(c, xwm):
                # step2: Z^T[f, d] = sum_s xws[s, f] * At[s, d]
                at = ac[c]
                ot = ot_pool.tile([P, fc, dcap], x_dt)
                last = c == cpc - 1
                dstep = 256 if last else 512
                ncast = 0
                for f in range(fc):
                    for d0 in range(0, dcap, dstep):
                        dn = min(dstep, dcap - d0)
                        ps = ps2_pool.tile([P, 512], f32, name="ps2t")
                        for st in range(sch):
                            h, t = divmod(st, 2)
                            nc.tensor.matmul(
                                ps[:, :dn],
                                lhsT=xwm[h][:, t, f * P:(f + 1) * P],
                                rhs=at[:, st * dcap + d0:st * dcap + d0 + dn],
                                start=(st == 0), stop=(st == sch - 1),
                            )
                        if ncast % 2 == 0 or c == 0:
                            eng = nc.vector.tensor_copy
                        else:
                            eng = nc.scalar.copy
                        eng(ot[:, f, d0:d0 + dn], ps[:, :dn])
                        ncast += 1
                    if last:
                        # tail: store each f-chunk as soon as it drains,
                        # on two queues so the issues run in parallel
                        eng = nc.sync if f == 0 else nc.scalar
                        eng.dma_start(YT[c][:, f], ot[:, f])
                if not last:
                    # one 250KB store per cluster on the scalar queue
                    nc.scalar.dma_start(YT[c][:], ot[:])

            def step1(c):
                xt = xc[c]
                xwm = []
                for h in range(hs):
                    ps = ps1_pool.tile([P, 2, f_out], f32, name="ps1t")
                    for t in range(2):
                        st = 2 * h + t
                        for k in range(kc):
                            nc.tensor.matmul(
                                ps[:, t],
                                lhsT=xt[:, k, st * P:(st + 1) * P],
                                rhs=wt[:, k],
                                start=(k == 0), stop=(k == kc - 1),
                            )
                    xw = xw_pool.tile([P, 2, f_out], x_dt, name="xwt")
                    if c < 2:
                        # scalar's first ACTIVATE is blocked on its cold
                        # activation-table DMA until ~11us -- keep it off
                        # the critical path for the first clusters
                        nc.vector.tensor_copy(xw[:], ps[:])
                    elif h == 0:
                        nc.scalar.copy(xw[:], ps[:])
                    else:
                        nc.vector.tensor_copy(xw[:], ps[:])
                    xwm.append(xw)
                return xwm

            # software pipeline: emit step1(c) before step2(c-1) so the PE
            # is never blocked on cluster-boundary casts
            pend = step1(0)
            for c in range(1, cpc):
                nxt = step1(c)
                step2(c - 1, pend)
                pend = nxt
            step2(cpc - 1, pend)

    nc.compile()
    _prog_cache[key] = nc
    return nc


def _host_prep(X, W, b, assign, full_ei):
    """Shard + preprocess. Returns (in_maps, a_fp8, gather info)."""
    n, in_c = X.shape
    f_out = W.shape[1]
    src = full_ei[0].astype(np.int64)
    dst = full_ei[1].astype(np.int64)
    a_s = assign[src]
    intra = a_s == assign[dst]
    es, ed = src[intra], dst[intra]

    deg = np.ones(n, np.float32)
    np.add.at(deg, ed, np.float32(1))
    dis = (1.0 / np.sqrt(deg)).astype(np.float32)

    has_edge = np.zeros(N_CLUSTERS, bool)
    has_edge[np.unique(a_s[intra])] = True

    sizes = np.bincount(assign, minlength=N_CLUSTERS)
    cpc = -(-N_CLUSTERS // N_CORES)                 # clusters per core
    cap = max(512, int(-(-sizes.max() // P)) * P)   # padded cluster size (s)
    dcap = int(sizes.max())                         # exact d extent
    sch = cap // P

    starts = np.zeros(N_CLUSTERS + 1, np.int64)
    starts[1:] = np.cumsum(sizes)
    order = np.argsort(assign, kind="stable")
    pos = np.empty(n, np.int64)
    pos[order] = np.arange(n) - starts[assign[order]]

    ctot = cpc * N_CORES
    # At blocks: At[c][s, d] = #edges(s->d) + [s==d]
    At = np.zeros((ctot, cap, dcap), np.uint16)
    np.add.at(At, (assign[es], pos[es], pos[ed]), 1)
    At[assign, pos, pos] += 1
    a_fp8 = int(At.max()) <= 16    # integers <= 16 are exact in e4m3

    if a_fp8:
        import concourse.mybir as mybir
        a_np = mybir.dt.np(mybir.dt.float8e4)
    else:
        a_np = np.float16
    # [c, s, d] -> [p, c, st*dcap + d] so each partition row is contiguous
    At_send = np.ascontiguousarray(
        At.astype(a_np).reshape(ctot, sch, P, dcap).transpose(2, 0, 1, 3)
    ).reshape(P, ctot, sch * dcap)

    # pre-scaled X, padded per cluster, partition-major with 2KB rows:
    # XT[p, c, k, j] = Xs[c, j, k*P + p]
    Xs = X.astype(np.float32) * dis[:, None]
    Xp = np.zeros((ctot, cap, in_c), np.float32)
    Xp[assign, pos] = Xs
    kc = in_c // P
    XT_all = np.ascontiguousarray(
        Xp.transpose(2, 0, 1).reshape(kc, P, ctot, cap)
        .transpose(1, 2, 0, 3)).astype(np.float16)

    WT_send = np.ascontiguousarray(
        W.astype(np.float32).reshape(kc, P, f_out).transpose(1, 0, 2)
    ).astype(np.float16)

    in_maps = []
    for i in range(N_CORES):
        xt_i = np.ascontiguousarray(XT_all[:, i * cpc:(i + 1) * cpc])
        # cluster 0's X with W packed into the row tails: [P, kc, cap+f_out]
        xw0_i = np.ascontiguousarray(
            np.concatenate([xt_i[:, 0], WT_send], axis=2))
        in_maps.append({
            "XT": xt_i,
            "XW0": xw0_i,
            "AT": np.ascontiguousarray(At_send[:, i * cpc:(i + 1) * cpc]),
        })
    return in_maps, a_fp8, (cpc, cap, dcap, has_edge, pos, dis)


def _run(inputs, trace=False, tmpdir=None):
    from concourse.bass_utils import run_bass_kernel_spmd

    X = np.asarray(inputs["X"], np.float32)
    W = np.asarray(inputs["W"], np.float32)
    b = np.asarray(inputs["b"], np.float32)
    assign = np.asarray(inputs["assign"])
    full_ei = np.asarray(inputs["full_ei"])

    n, in_c = X.shape
    f_out = W.shape[1]
    in_maps, a_fp8, (cpc, cap, dcap, has_edge, pos, dis) = _host_prep(
        X, W, b, assign, full_ei)
    nc = _build_program(cpc, cap, dcap, in_c, f_out, a_fp8)

    res = run_bass_kernel_spmd(
        nc, in_maps, core_ids=list(range(N_CORES)),
        trace=trace, tmpdir=tmpdir,
    )
    # YT: [core][cpc, P, fc, dcap]; Y[n, fi*P + p] = YT[core, lc, p, fi, pos]
    YTdev = np.stack([res.results[i]["YT"] for i in range(N_CORES)])
    if YTdev.dtype != np.float32:
        YTdev = YTdev.astype(np.float32)
    fc = f_out // P
    Yt = YTdev.transpose(0, 1, 3, 2, 4).reshape(N_CORES, cpc, f_out, dcap)

    c = assign.astype(np.int64)
    core = c // cpc
    lc = c % cpc
    Y = Yt[core, lc, :, pos]
    Y *= dis[:, None]
    Y += b[None, :].astype(np.float32)
    miss = ~has_edge[c]
    if miss.any():
        Y[miss] = X[miss]
    return Y, res


def kernel(**inputs) -> np.ndarray:
    Y, _ = _run(inputs)
    return Y
